# revision 1
# baseline (speedup 1.0000x reference)
"""BoxCrop kernel for Trainium2 (8 NeuronCores, Bass/Tile).

Fused crop -> aspect-preserving bilinear resize (long side 336) -> square pad
(fill=127) for a batch of 64 images [64,3,768,768] with per-image XYWH boxes.

Strategy (pure data-parallel, 8 images per core):
- Host computes, per image: crop-row gather offsets (128-element-block
  indices into the image shard viewed as [n_blocks, 128]; each descriptor
  reads 512 contiguous elements = 4 blocks covering image columns
  [128*(xb//128), +512) which always contains the crop columns), clamped
  crop-local source coordinates (syc for rows; sxc shifted by the xb%128
  residual for columns; -1e6 for pad rows/cols), and pad-fill params.
- Device per image-channel:
    crop[r, 0:512] via indirect-DMA row gather (f32r, 512B+ descriptors);
    A'[r,i]  = -relu(1-|r-syc_i|)   r in [0,384)   (3 chunks)
    Wx'[k,j] = -relu(1-|k-sxc_j|)   k in [0,512)   (4 chunks)
    (negated tents == exact bilinear weights incl. boundary clamping;
     the two negations cancel in the product)
    RT = crop^T @ A'  (f32r matmuls, contraction over r, partition = k);
    M  = RT-contraction with Wx' (over k) accumulated in PSUM;
    out = M + (127 - 127*vy_i*vx_j)  (pad fill) during PSUM->SBUF.
"""
import numpy as np

import concourse.bacc as bacc
import concourse.bass as bass
import concourse.tile as tile
from concourse import mybir
from concourse.bass import AP, IndirectOffsetOnAxis
from concourse.bass_utils import run_bass_kernel_spmd

F32 = mybir.dt.float32
F32R = mybir.dt.float32r
I32 = mybir.dt.int32

N_CORES = 8
B = 64
BL = B // N_CORES          # images per core
C = 3
H = W = 768
O = 336                    # output size
RROWS = 384                # gathered crop rows (static max)
KCOLS = 512                # gathered columns per row (4 x 128-elem blocks)
PLANE = H * W
IMG_ELEMS = C * PLANE
TOT_ELEMS = BL * IMG_ELEMS
BLK = 128
N_BLOCKS = TOT_ELEMS // BLK
ROW_BLKS = W // BLK        # 6 blocks per image row
BOUND = N_BLOCKS - KCOLS // BLK   # max valid gather start block
BIG = 1 << 27
FILL = 127.0
CROP_BUFS = 3              # first CROP_BUFS images must fully write their tile

_CACHED = None
LAST_RESULT = None


def _build(reps: int = 1):
    nc = bacc.Bacc("TRN2", target_bir_lowering=False, debug=False)

    imgs = nc.dram_tensor("imgs", [N_BLOCKS, BLK], F32R, kind="ExternalInput")
    offs = nc.dram_tensor("offs", [128, BL * 9], I32, kind="ExternalInput")
    par_row = nc.dram_tensor("par_row", [128, BL * 672], F32, kind="ExternalInput")
    par_col = nc.dram_tensor("par_col", [128, BL * 3], F32, kind="ExternalInput")
    iota = nc.dram_tensor("iota", [128, 4], F32, kind="ExternalInput")
    out = nc.dram_tensor("out", [BL, C, O, O], F32, kind="ExternalOutput")

    with tile.TileContext(nc) as tc:
        with (
            tc.tile_pool(name="const", bufs=1) as cpool,
            tc.tile_pool(name="crop", bufs=CROP_BUFS) as crop_pool,
            tc.tile_pool(name="tent", bufs=2) as tent_pool,
            tc.tile_pool(name="dtmp", bufs=3) as dtmp_pool,
            tc.tile_pool(name="fill", bufs=2) as fill_pool,
            tc.tile_pool(name="rt", bufs=3) as rt_pool,
            tc.tile_pool(name="osb", bufs=2) as out_pool,
            tc.tile_pool(name="ps1", bufs=4, space="PSUM") as ps1,
            tc.tile_pool(name="ps2", bufs=4, space="PSUM") as ps2,
        ):
            offs_sb = cpool.tile([128, BL * 9], I32, tag="offs")
            nc.sync.dma_start(offs_sb[:], offs[:])
            par_sb = cpool.tile([128, BL * 672], F32, tag="par")
            nc.sync.dma_start(par_sb[:], par_row[:])
            parcol_sb = cpool.tile([128, BL * 3], F32, tag="parcol")
            nc.sync.dma_start(parcol_sb[:], par_col[:])
            iota_sb = cpool.tile([128, 4], F32, tag="iota")
            nc.sync.dma_start(iota_sb[:], iota[:])

            for b in range(reps * BL):
                b = b % BL
                bc = par_sb[:, b * 672 : (b + 1) * 672]

                # negated tents, f32r. tent slots: 2t   = A' chunk t (t<3)
                #                                  2t+1 = Wx' chunk t (t<3)
                #                                  6    = Wx' chunk 3
                tent = tent_pool.tile([128, 7, 336], F32R, tag="tent")
                for t in range(3):
                    dtmp = dtmp_pool.tile([128, 672], F32, tag="dtmp")
                    nc.scalar.activation(
                        dtmp[:],
                        bc[:, 0:672],
                        mybir.ActivationFunctionType.Abs,
                        bias=iota_sb[:, t : t + 1],
                        scale=-1.0,
                    )
                    nc.vector.tensor_scalar(
                        out=tent[:, 2 * t : 2 * t + 2, :].rearrange("p a b -> p (a b)"),
                        in0=dtmp[:],
                        scalar1=1.0,
                        scalar2=0.0,
                        op0=mybir.AluOpType.subtract,
                        op1=mybir.AluOpType.min,
                    )
                dtmp = dtmp_pool.tile([128, 672], F32, tag="dtmp")
                nc.scalar.activation(
                    dtmp[:, 0:336],
                    bc[:, 336:672],
                    mybir.ActivationFunctionType.Abs,
                    bias=iota_sb[:, 3:4],
                    scale=-1.0,
                )
                nc.vector.tensor_scalar(
                    out=tent[:, 6, :],
                    in0=dtmp[:, 0:336],
                    scalar1=1.0,
                    scalar2=0.0,
                    op0=mybir.AluOpType.subtract,
                    op1=mybir.AluOpType.min,
                )

                # pad fill: 127 - 127*vy_i*vx_j == vx*(-127*vy) + 127
                # vx reconstructed on device: vx = (sxc >= -1e5)
                fill = fill_pool.tile([112, 4, 336], F32, tag="fill")
                nc.vector.tensor_scalar(
                    out=fill[:, 3, :],
                    in0=bc[0:112, 336:672],
                    scalar1=-1e5,
                    scalar2=None,
                    op0=mybir.AluOpType.is_ge,
                )
                for ic in range(3):
                    nc.vector.tensor_scalar(
                        out=fill[:, ic, :],
                        in0=fill[:, 3, :],
                        scalar1=parcol_sb[0:112, b * 3 + ic : b * 3 + ic + 1],
                        scalar2=FILL,
                        op0=mybir.AluOpType.mult,
                        op1=mybir.AluOpType.add,
                    )

                # gather crop rows (slot 3c+t holds crop rows 128t..128t+127)
                crop = crop_pool.tile([128, 9, KCOLS], F32R, tag="crop")
                for s in range(9):
                    col = b * 9 + s
                    nc.gpsimd.indirect_dma_start(
                        out=crop[:, s, :],
                        out_offset=None,
                        in_=imgs[:, :],
                        in_offset=IndirectOffsetOnAxis(
                            ap=offs_sb[:, col : col + 1], axis=0
                        ),
                        bounds_check=BOUND,
                        oob_is_err=False,
                    )

                out_sb = out_pool.tile([112, 9, 336], F32, tag="osb")
                for c in range(C):
                    rt = rt_pool.tile([128, 4, 336], F32R, tag="rt")
                    for k2 in range(4):
                        pmm = ps1.tile([128, 336], F32, tag="pmm")
                        for t in range(3):
                            nc.tensor.matmul(
                                pmm[:],
                                crop[:, 3 * c + t, 128 * k2 : 128 * (k2 + 1)],
                                tent[:, 2 * t, :],
                                start=(t == 0),
                                stop=(t == 2),
                            )
                        nc.scalar.copy(rt[:, k2, :], pmm[:])
                    for ic in range(3):
                        pm2 = ps2.tile([112, 336], F32, tag="pm2")
                        for k2 in range(4):
                            nc.tensor.matmul(
                                pm2[:],
                                rt[:, k2, 112 * ic : 112 * (ic + 1)],
                                tent[:, 2 * k2 + 1 if k2 < 3 else 6, :],
                                start=(k2 == 0),
                                stop=(k2 == 3),
                            )
                        nc.vector.tensor_tensor(
                            out=out_sb[:, 3 * c + ic, :],
                            in0=pm2[:],
                            in1=fill[:, ic, :],
                            op=mybir.AluOpType.add,
                        )

                    # store channel c: [112, 3, 336] -> out[b, c]
                    dst = AP(
                        tensor=out,
                        offset=(b * C + c) * O * O,
                        ap=[[O, 112], [112 * O, 3], [1, O]],
                    )
                    nc.sync.dma_start(dst, out_sb[:, 3 * c : 3 * c + 3, :])

    nc.compile()
    return nc


def _host_params(images, boxes):
    """Per-core host prep. images: [BL,3,768,768] f32, boxes: [BL,4] i32."""
    f32 = np.float32
    offs = np.full((128, BL * 9), BIG, np.int32)
    par_rows = np.empty((1, BL * 672), np.float32)  # broadcast at end
    par_cols = np.zeros((128, BL * 3), np.float32)

    grid = np.arange(O, dtype=np.int64)
    for b in range(BL):
        xb, yb, wb, hb = (int(v) for v in boxes[b])
        wf, hf = f32(wb), f32(hb)
        scale = f32(O) / np.maximum(wf, hf)
        new_w = int(np.round(wf * scale))
        new_h = int(np.round(hf * scale))
        pad_top = (O - new_h) // 2 if hb < wb else 0
        pad_left = (O - new_w) // 2 if hb >= wb else 0

        def axis_params(pad, new_n, nf, lim):
            i = grid - pad
            valid = (i >= 0) & (i < new_n)
            src = (i.astype(f32) + f32(0.5)) * nf
            src = src / f32(new_n)
            src = src - f32(0.5)        # crop-local source coordinate
            src = np.clip(src, f32(0.0), f32(lim - 1))
            src[~valid] = f32(-1e6)
            return src.astype(np.float32), valid.astype(np.float32)

        syc, vy = axis_params(pad_top, new_h, hf, hb)
        sxc, vx = axis_params(pad_left, new_w, wf, wb)
        # shift column coords by the xb%128 residual of the gather window
        x_shift = f32(xb - BLK * (xb // BLK))
        sxc = np.where(sxc > f32(-1e5), sxc + x_shift, sxc).astype(np.float32)

        par_rows[0, b * 672 : b * 672 + 336] = syc
        par_rows[0, b * 672 + 336 : b * 672 + 672] = sxc
        for ic in range(3):
            par_cols[0:112, b * 3 + ic] = -FILL * vy[ic * 112 : (ic + 1) * 112]

        # gather offsets (128-elem block indices): slot s = 3c+t,
        # partition p -> crop row 128t+p
        p = np.arange(128)
        xblk = xb // BLK
        for c in range(C):
            for t in range(3):
                r = 128 * t + p
                rr = np.minimum(r, hb - 1)
                off = (b * IMG_ELEMS + c * PLANE) // BLK + (yb + rr) * ROW_BLKS + xblk
                if b >= CROP_BUFS:
                    off = np.where(r < hb, off, BIG)
                offs[:, b * 9 + 3 * c + t] = off.astype(np.int32)

    iota = (np.arange(128)[:, None] + 128 * np.arange(4)[None, :]).astype(np.float32)
    return dict(
        imgs=np.ascontiguousarray(images).reshape(N_BLOCKS, BLK),
        offs=offs,
        par_row=np.ascontiguousarray(np.broadcast_to(par_rows, (128, BL * 672))),
        par_col=par_cols,
        iota=iota,
    )


def kernel(images: np.ndarray, boxes: np.ndarray) -> np.ndarray:
    global _CACHED, LAST_RESULT
    if _CACHED is None:
        _CACHED = _build()
    nc = _CACHED

    in_maps = [
        _host_params(
            np.asarray(images[m * BL : (m + 1) * BL], dtype=np.float32),
            np.asarray(boxes[m * BL : (m + 1) * BL]),
        )
        for m in range(N_CORES)
    ]
    res = run_bass_kernel_spmd(nc, in_maps, core_ids=list(range(N_CORES)))
    LAST_RESULT = res
    return np.concatenate([r["out"] for r in res.results], axis=0)



# revision 20
# speedup vs baseline: 1.5071x; 1.5071x over previous
"""BoxCrop kernel for Trainium2 (8 NeuronCores, Bass/Tile).

Fused crop -> aspect-preserving bilinear resize (long side 336) -> square pad
(fill=127) for a batch of 64 images [64,3,768,768] with per-image XYWH boxes.

Strategy (pure data-parallel, 8 images per core), v3 "windowed fp16, HWC":
- Host converts images to fp16 in channel-interleaved [H,W,C] layout and
  computes, per image: crop-local bilinear source coords for both axes (f32,
  clamped, invalid = -30000), row window starts rs_t for each 112-row output
  block (bilinear slope <= ~8/7 so each block sources <= 128 consecutive
  crop rows -- asserted on the data), per-window gather offsets, and uint8
  quantization params.
- Device per image:
    coords: rank-1 bf16 matmuls (hi+lo split, exact to ~2^-18) broadcast
      [1,336] -> PSUM [128,336] per slot (sy_w | sx | sx-128 | sx-256).
    tents: Abs activation (bias=iota_p, scale=-1) + tensor_scalar
      min(d-1,0) -> negated tents fp16 [128,4,336]; slot 0 = vertical A'
      (window-relative), slots 1-3 = horizontal Wx col-chunks.
    gather: 3 indirect DMAs (one per row block), offsets [128,1] (one
      descriptor per partition, the only HW-supported form), each
      descriptor = 1152 contiguous elems (384 cols x 3 channels, 2304B).
    stage 1 (vertical): per (ch, col-chunk q): 3 matmuls, contraction =
      the 128-row window of each block, lhsT = stride-3 channel view of
      the gathered rows, N=112 -> PSUM [128,336]; copy to SBUF fp16.
    stage 2 (horizontal): per (ch, i-tile T): 3 accumulating matmuls over
      col-chunks (lhsT = RT[:,q,112T:+112], rhs = Wx_q, N=336) -> PSUM
      [112,336]; quantize-copy (scale,bias) -> uint8 out tile.
    out DMA per image: [112,9,336] uint8 -> DRAM (1008B descriptors);
      host dequantizes, un-permutes rows, and fills the 127 pad region.
- (negated tents: the stage-1 and stage-2 negations cancel in the product)
"""
import numpy as np

import concourse.bacc as bacc
import concourse.bass as bass
import concourse.tile as tile
from concourse import mybir
from concourse.bass import AP, IndirectOffsetOnAxis
from concourse.bass_utils import run_bass_kernel_spmd

F32 = mybir.dt.float32
BF16 = mybir.dt.bfloat16
F16 = mybir.dt.float16
U8 = mybir.dt.uint8
I32 = mybir.dt.int32

N_CORES = 8
B = 64
BL = B // N_CORES          # images per core
C = 3
H = W = 768
O = 336                    # output size
BI = 112                   # output rows per block (3 blocks)
KC = 384                   # gathered columns per row window
ROWE = KC * C              # elements per descriptor (cols x channels)
IMG_ELEMS = C * H * W
TOT = BL * IMG_ELEMS
INVALID = np.float32(-30000.0)

_CACHED = None
LAST_RESULT = None


def _build():
    nc = bacc.Bacc("TRN2", target_bir_lowering=False, debug=False)

    imgs = nc.dram_tensor("imgs", [1, TOT], F16, kind="ExternalInput")
    offs = nc.dram_tensor("offs", [128, BL * 3], I32, kind="ExternalInput")
    chi = nc.dram_tensor("chi", [1, BL * 4 * O], BF16, kind="ExternalInput")
    clo = nc.dram_tensor("clo", [1, BL * 4 * O], BF16, kind="ExternalInput")
    qp = nc.dram_tensor("qp", [128, 2], F32, kind="ExternalInput")
    ones = nc.dram_tensor("ones", [1, 128], BF16, kind="ExternalInput")
    iota = nc.dram_tensor("iota", [128, 1], F32, kind="ExternalInput")
    out = nc.dram_tensor("out", [BL, C, BI, 3, O], U8, kind="ExternalOutput")

    with tile.TileContext(nc) as tc:
        with (
            tc.tile_pool(name="const", bufs=1) as cpool,
            tc.tile_pool(name="crop", bufs=2) as crop_pool,
            tc.tile_pool(name="dtmp", bufs=2) as dtmp_pool,
            tc.tile_pool(name="tent", bufs=2) as tent_pool,
            tc.tile_pool(name="rt", bufs=3) as rt_pool,
            tc.tile_pool(name="osb", bufs=2) as out_pool,
            tc.tile_pool(name="pc", bufs=1, space="PSUM") as pc_pool,
            tc.tile_pool(name="ps1", bufs=3, space="PSUM") as ps1,
            tc.tile_pool(name="ps2", bufs=3, space="PSUM") as ps2,
        ):
            # spread the const loads over both HWDGE queues, tent inputs first
            chi_sb = cpool.tile([1, BL * 4 * O], BF16, tag="chi")
            nc.sync.dma_start(chi_sb[:], chi[:])
            clo_sb = cpool.tile([1, BL * 4 * O], BF16, tag="clo")
            nc.scalar.dma_start(clo_sb[:], clo[:])
            ones_sb = cpool.tile([1, 128], BF16, tag="ones")
            nc.scalar.dma_start(ones_sb[:], ones[:])
            iota_sb = cpool.tile([128, 1], F32, tag="iota")
            nc.scalar.dma_start(iota_sb[:], iota[:])
            offs_sb = cpool.tile([128, BL * 3], I32, tag="offs")
            nc.sync.dma_start(offs_sb[:], offs[:])
            qp_sb = cpool.tile([128, 2], F32, tag="qp")
            nc.sync.dma_start(qp_sb[:], qp[:])

            # PSUM->SBUF copies may only run on Act or DVE (GPSIMD cannot
            # access PSUM); Act also runs the Abs pass, so DVE gets more
            cp_engines = [1, 0, 1, 1, 0]  # 0=Act 1=DVE
            cp_idx = 0

            def copy_rot(dst, src, quant):
                nonlocal cp_idx
                e = cp_engines[cp_idx % len(cp_engines)]
                cp_idx += 1
                if quant:
                    scale = qp_sb[0:BI, 0:1]
                    bias = qp_sb[0:BI, 1:2]
                    if e == 0:
                        # quantized values are all > 0, so Relu == identity
                        # (Copy does not accept AP bias/scale)
                        nc.scalar.activation(
                            dst, src, mybir.ActivationFunctionType.Relu,
                            bias=bias, scale=scale,
                        )
                    else:
                        nc.vector.tensor_scalar(
                            out=dst, in0=src, scalar1=scale, scalar2=bias,
                            op0=mybir.AluOpType.mult, op1=mybir.AluOpType.add,
                        )
                else:
                    if e == 0:
                        nc.scalar.copy(dst, src)
                    else:
                        nc.vector.tensor_copy(dst, src)

            def make_tents(b):
                """Broadcast coords (bf16 hi+lo rank-1 matmuls), Abs, tent."""
                tent = tent_pool.tile([128, 4, O], F16, tag="tent")
                dtmp = dtmp_pool.tile([128, 4, O], F16, tag="dtmp")
                for h in range(2):
                    pc = pc_pool.tile([128, 2, 512], F32, tag="pc")
                    for s in range(2):
                        sl = (b * 4 + 2 * h + s) * O
                        nc.tensor.matmul(
                            pc[:, s, 0:O], ones_sb[:],
                            chi_sb[:, sl : sl + O],
                            start=True, stop=False,
                        )
                        nc.tensor.matmul(
                            pc[:, s, 0:O], ones_sb[:],
                            clo_sb[:, sl : sl + O],
                            start=False, stop=True,
                        )
                    nc.scalar.activation(
                        dtmp[:, 2 * h : 2 * h + 2, :],
                        pc[:, :, 0:O],
                        mybir.ActivationFunctionType.Abs,
                        bias=iota_sb[:, 0:1],
                        scale=-1.0,
                    )
                nc.vector.tensor_scalar(
                    out=tent[:],
                    in0=dtmp[:],
                    scalar1=1.0,
                    scalar2=0.0,
                    op0=mybir.AluOpType.subtract,
                    op1=mybir.AluOpType.min,
                )
                return tent

            def start_gather(b):
                # one indirect DMA per row-window block; offsets [128,1] is
                # the only descriptor layout real SWDGE supports
                crop = crop_pool.tile([128, 3, ROWE], F16, tag="crop")
                for t in range(3):
                    nc.gpsimd.indirect_dma_start(
                        out=crop[:, t, :],
                        out_offset=None,
                        in_=imgs[:, :],
                        in_offset=IndirectOffsetOnAxis(
                            ap=offs_sb[:, b * 3 + t : b * 3 + t + 1], axis=1
                        ),
                    )
                # stride-3 channel view: [128, block, ch, col]
                return crop, crop.rearrange("p t (x c) -> p t c x", c=C)

            # software pipeline: tents and gather run one image ahead
            tent_next = make_tents(0)
            crop_next = start_gather(0)
            for b in range(BL):
                tent, (crop, cview) = tent_next, crop_next
                if b + 1 < BL:
                    crop_next = start_gather(b + 1)
                    tent_next = make_tents(b + 1)

                out_sb = out_pool.tile([BI, 9, O], U8, tag="osb")

                def stage1(c):
                    # vertical resize, row-windowed
                    rt = rt_pool.tile([128, 3, O], F16, tag="rt")
                    for q in range(3):
                        pmm = ps1.tile([128, O], F32, tag="pmm")
                        for t in range(3):
                            nc.tensor.matmul(
                                pmm[:, BI * t : BI * (t + 1)],
                                cview[:, t, c, 128 * q : 128 * (q + 1)],
                                tent[:, 0, BI * t : BI * (t + 1)],
                                start=True, stop=True,
                            )
                        copy_rot(rt[:, q, :], pmm[:], quant=False)
                    return rt

                def stage2(c, rt):
                    # horizontal resize + quantize
                    for T in range(3):
                        pm2 = ps2.tile([BI, O], F32, tag="pm2")
                        for q in range(3):
                            nc.tensor.matmul(
                                pm2[:],
                                rt[:, q, BI * T : BI * (T + 1)],
                                tent[:, 1 + q, :],
                                start=(q == 0), stop=(q == 2),
                            )
                        copy_rot(out_sb[:, 3 * c + T, :], pm2[:], quant=True)

                # interleave: stage1 of channel c+1 issues before stage2 of
                # channel c, so PE has independent work while the rt copies
                # for channel c drain through the vector engines
                rt_c = stage1(0)
                for c in range(C):
                    rt_n = stage1(c + 1) if c + 1 < C else None
                    stage2(c, rt_c)
                    rt_c = rt_n

                dst = AP(
                    tensor=out,
                    offset=b * C * BI * 3 * O,
                    ap=[[3 * O, BI], [BI * 3 * O, C], [O, 3], [1, O]],
                )
                nc.sync.dma_start(dst, out_sb[:])

    nc.compile()
    return nc


def _axis_coords(pad, new_n, nf, lim):
    """Crop-local bilinear source coords, f32 math mirroring the reference."""
    f32 = np.float32
    i = np.arange(O, dtype=np.int64) - pad
    valid = (i >= 0) & (i < new_n)
    src = (i.astype(f32) + f32(0.5)) * nf
    src = src / f32(new_n)
    src = src - f32(0.5)
    src = np.clip(src, f32(0.0), f32(lim - 1))
    src[~valid] = INVALID
    return src.astype(np.float32), valid


def _host_params(images, boxes):
    """Per-core host prep. images: [BL,3,768,768] f32, boxes: [BL,4] i32.

    Returns (input map, per-image pad/dequant info).
    """
    import ml_dtypes

    f32 = np.float32
    offs = np.zeros((128, BL * 3), np.int32)
    coords = np.full((BL * 4 * O,), INVALID, np.float32)

    mn = float(images.min()) - 2.0
    mx = float(images.max()) + 2.0
    qa = (mx - mn) / 255.0
    qb = mn
    qp = np.zeros((128, 2), np.float32)
    qp[:, 0] = 1.0 / qa
    qp[:, 1] = -qb / qa + 0.5  # +0.5: convert-to-uint8 truncates

    info = []
    p = np.arange(128)
    for b in range(BL):
        xb, yb, wb, hb = (int(v) for v in boxes[b])
        wf, hf = f32(wb), f32(hb)
        scale = f32(O) / np.maximum(wf, hf)
        new_w = int(np.round(wf * scale))
        new_h = int(np.round(hf * scale))
        pad_top = (O - new_h) // 2 if hb < wb else 0
        pad_left = (O - new_w) // 2 if hb >= wb else 0

        sy, vy = _axis_coords(pad_top, new_h, hf, hb)
        sx, vx = _axis_coords(pad_left, new_w, wf, wb)

        # per-block row windows: window [base_t, base_t+128) of IMAGE rows,
        # guaranteed inside the image, covering all taps of the block
        syw = sy.copy()
        for t in range(3):
            blk = slice(BI * t, BI * (t + 1))
            v = vy[blk]
            base = 0
            if v.any():
                s = sy[blk][v]
                lo = int(np.floor(s.min()))
                hi = min(int(np.floor(s.max())) + 1, hb - 1)
                assert hi - lo <= 127, (b, t, lo, hi)
                base = min(yb + lo, H - 128)
                syw[blk] = np.where(v, sy[blk] + f32(yb - base), INVALID)
            else:
                base = min(yb, H - 128)
            offs[:, b * 3 + t] = (((base + p) * W + xb) * C
                                  + b * IMG_ELEMS).astype(np.int32)

        base4 = b * 4 * O
        coords[base4 : base4 + O] = syw
        coords[base4 + O : base4 + 2 * O] = sx
        coords[base4 + 2 * O : base4 + 3 * O] = np.where(
            vx, sx - f32(128.0), INVALID
        )
        coords[base4 + 3 * O : base4 + 4 * O] = np.where(
            vx, sx - f32(256.0), INVALID
        )

        info.append((pad_top, new_h, pad_left, new_w))

    chi = coords.astype(ml_dtypes.bfloat16)
    clo = (coords - chi.astype(np.float32)).astype(ml_dtypes.bfloat16)

    in_map = dict(
        imgs=np.ascontiguousarray(
            images.transpose(0, 2, 3, 1)
        ).astype(np.float16).reshape(1, TOT),
        offs=offs,
        chi=chi.reshape(1, -1),
        clo=clo.reshape(1, -1),
        qp=qp,
        ones=np.ones((1, 128), ml_dtypes.bfloat16),
        iota=np.arange(128, dtype=np.float32).reshape(128, 1),
    )
    return in_map, (qa, qb, info)


def kernel(images: np.ndarray, boxes: np.ndarray) -> np.ndarray:
    global _CACHED, LAST_RESULT
    if _CACHED is None:
        _CACHED = _build()
    nc = _CACHED

    prep = [
        _host_params(
            np.asarray(images[m * BL : (m + 1) * BL], dtype=np.float32),
            np.asarray(boxes[m * BL : (m + 1) * BL]),
        )
        for m in range(N_CORES)
    ]
    in_maps = [pm for pm, _ in prep]
    res = run_bass_kernel_spmd(nc, in_maps, core_ids=list(range(N_CORES)))
    LAST_RESULT = res

    full = np.empty((B, C, O, O), np.float32)
    for m in range(N_CORES):
        qa, qb, info = prep[m][1]
        raw = np.asarray(res.results[m]["out"])  # [BL, C, 112, 3, 336] uint8
        deq = raw.astype(np.float32) * np.float32(qa) + np.float32(qb)
        # row index i = 112*ic + p  ->  [BL, C, 336, 336]
        deq = deq.transpose(0, 1, 3, 2, 4).reshape(BL, C, O, O)
        for b in range(BL):
            pt, nh, pl, nw = info[b]
            g = m * BL + b
            full[g] = np.float32(127.0)
            full[g, :, pt : pt + nh, pl : pl + nw] = deq[
                b, :, pt : pt + nh, pl : pl + nw
            ]
    return full


# revision 47
# speedup vs baseline: 1.6592x; 1.1010x over previous
"""BoxCrop kernel for Trainium2 (8 NeuronCores, Bass/Tile).

Fused crop -> aspect-preserving bilinear resize (long side 336) -> square pad
(fill=127) for a batch of 64 images [64,3,768,768] with per-image XYWH boxes.

Strategy (pure data-parallel, 8 images per core), v3 "windowed fp16, HWC":
- Host converts images to fp16 in channel-interleaved [H,W,C] layout and
  computes, per image: crop-local bilinear source coords for both axes (f32,
  clamped, invalid = -30000), row window starts rs_t for each 112-row output
  block (bilinear slope <= ~8/7 so each block sources <= 128 consecutive
  crop rows -- asserted on the data), per-window gather offsets, and uint8
  quantization params.
- Device per image:
    coords: rank-1 bf16 matmuls (hi+lo split, exact to ~2^-18) broadcast
      [1,336] -> PSUM [128,336] per slot (sy_w | sx | sx-128 | sx-256).
    tents: Abs activation (bias=iota_p, scale=-1) + tensor_scalar
      min(d-1,0) -> negated tents fp16 [128,4,336]; slot 0 = vertical A'
      (window-relative), slots 1-3 = horizontal Wx col-chunks.
    gather: 3 indirect DMAs (one per row block), offsets [128,1] (one
      descriptor per partition, the only HW-supported form), each
      descriptor = 1152 contiguous elems (384 cols x 3 channels, 2304B).
    stage 1 (vertical): per (ch, col-chunk q): 3 matmuls, contraction =
      the 128-row window of each block, lhsT = stride-3 channel view of
      the gathered rows, N=112 -> PSUM [128,336]; copy to SBUF fp16.
    stage 2 (horizontal): per (ch, i-tile T): 3 accumulating matmuls over
      col-chunks (lhsT = RT[:,q,112T:+112], rhs = Wx_q, N=336) -> PSUM
      [112,336]; quantize-copy (scale,bias) -> uint8 out tile.
    out DMA per image: [112,9,336] uint8 -> DRAM (1008B descriptors);
      host dequantizes, un-permutes rows, and fills the 127 pad region.
- (negated tents: the stage-1 and stage-2 negations cancel in the product)
"""
import numpy as np

import concourse.bacc as bacc
import concourse.bass as bass
import concourse.tile as tile
from concourse import mybir
from concourse.bass import AP, IndirectOffsetOnAxis
from concourse.bass_utils import run_bass_kernel_spmd

F32 = mybir.dt.float32
F32R = mybir.dt.float32r
BF16 = mybir.dt.bfloat16
F16 = mybir.dt.float16
U8 = mybir.dt.uint8
I32 = mybir.dt.int32

N_CORES = 8
B = 64
BL = B // N_CORES          # images per core
C = 3
H = W = 768
O = 336                    # output size
BI = 112                   # output rows per block (3 blocks)
KC = 384                   # gathered columns per row window
ROWE = KC * C              # elements per descriptor (cols x channels)
IMG_ELEMS = C * H * W
TOT = BL * IMG_ELEMS
INVALID = np.float32(-30000.0)

_CACHED = None
LAST_RESULT = None


def _build():
    nc = bacc.Bacc("TRN2", target_bir_lowering=False, debug=False)

    imgs = nc.dram_tensor("imgs", [1, TOT], F16, kind="ExternalInput")
    # pk1: offs[0:24] | iota(f32 bits)[24:27] | qp(f32 bits)[27:29]
    pk1 = nc.dram_tensor("pk1", [128, BL * 3 + 5], I32, kind="ExternalInput")
    # pk2: chi | clo | ones  (all bf16)
    NCO = BL * 2 * O
    pk2 = nc.dram_tensor("pk2", [1, 2 * NCO + 128], BF16, kind="ExternalInput")
    out = nc.dram_tensor("out", [BL, C, BI, 3, O], U8, kind="ExternalOutput")

    with tile.TileContext(nc) as tc:
        with (
            tc.tile_pool(name="const", bufs=1) as cpool,
            tc.tile_pool(name="crop", bufs=2) as crop_pool,
            tc.tile_pool(name="dtmp", bufs=2) as dtmp_pool,
            tc.tile_pool(name="tent", bufs=2) as tent_pool,
            tc.tile_pool(name="rt", bufs=3) as rt_pool,
            tc.tile_pool(name="osb", bufs=2) as out_pool,
            tc.tile_pool(name="pc", bufs=1, space="PSUM") as pc_pool,
            tc.tile_pool(name="ps1", bufs=3, space="PSUM") as ps1,
            tc.tile_pool(name="ps2", bufs=3, space="PSUM") as ps2,
        ):
            # two packed const loads, one per HWDGE queue, so the gather
            # chain (pk1/offs on SP) and the tent chain (pk2 on Act) start
            # in parallel with minimal head latency
            pk1_sb = cpool.tile([128, BL * 3 + 5], I32, tag="pk1")
            nc.sync.dma_start(pk1_sb[:], pk1[:])
            pk2_sb = cpool.tile([1, 2 * NCO + 128], BF16, tag="pk2")
            nc.scalar.dma_start(pk2_sb[:], pk2[:])
            NO3 = BL * 3

            def off_ap(col):
                return pk1_sb[:, col : col + 1]

            def iota_ap(q):
                return pk1_sb[:, NO3 + q : NO3 + q + 1].bitcast(F32)

            q_scale = pk1_sb[0:BI, NO3 + 3 : NO3 + 4].bitcast(F32)
            q_bias = pk1_sb[0:BI, NO3 + 4 : NO3 + 5].bitcast(F32)

            def chi_ap(sl):
                return pk2_sb[:, sl : sl + O]

            def clo_ap(sl):
                return pk2_sb[:, NCO + sl : NCO + sl + O]

            ones_sb = pk2_sb[:, 2 * NCO : 2 * NCO + 128]

            # PSUM->SBUF copies may only run on Act or DVE (GPSIMD cannot
            # access PSUM); Act also runs the Abs pass, so DVE gets more
            cp_engines = [1, 0, 1, 0, 1, 0, 1, 0, 1]  # 0=Act 1=DVE
            cp_idx = 0

            def copy_rot(dst, src, quant):
                nonlocal cp_idx
                e = cp_engines[cp_idx % len(cp_engines)]
                cp_idx += 1
                if quant:
                    scale = q_scale
                    bias = q_bias
                    if e == 0:
                        # quantized values are all > 0, so Relu == identity
                        # (Copy does not accept AP bias/scale)
                        nc.scalar.activation(
                            dst, src, mybir.ActivationFunctionType.Relu,
                            bias=bias, scale=scale,
                        )
                    else:
                        nc.vector.tensor_scalar(
                            out=dst, in0=src, scalar1=scale, scalar2=bias,
                            op0=mybir.AluOpType.mult, op1=mybir.AluOpType.add,
                        )
                else:
                    if e == 0:
                        nc.scalar.copy(dst, src)
                    else:
                        nc.vector.tensor_copy(dst, src)

            def make_tents(b):
                """Broadcast sy/sx coords (bf16 hi+lo rank-1 matmuls), then
                Abs with shifted iota biases (p, p+128, p+256) to derive all
                three Wx col-chunks from the single sx broadcast."""
                tent = tent_pool.tile([128, 4, O], F16, tag="tent")
                dtmp = dtmp_pool.tile([128, 4, O], F16, tag="dtmp")
                pc = pc_pool.tile([128, 2, 512], F32, tag="pc")
                for s in range(2):
                    sl = (b * 2 + s) * O
                    nc.tensor.matmul(
                        pc[:, s, 0:O], ones_sb,
                        chi_ap(sl),
                        start=True, stop=False,
                    )
                    nc.tensor.matmul(
                        pc[:, s, 0:O], ones_sb,
                        clo_ap(sl),
                        start=False, stop=True,
                    )
                # dtmp slots: 0 = |p - sy|, 1+q = |(128q + p) - sx|
                nc.scalar.activation(
                    dtmp[:, 0:2, :],
                    pc[:, :, 0:O],
                    mybir.ActivationFunctionType.Abs,
                    bias=iota_ap(0),
                    scale=-1.0,
                )
                for q in (1, 2):
                    nc.scalar.activation(
                        dtmp[:, 1 + q, :],
                        pc[:, 1, 0:O],
                        mybir.ActivationFunctionType.Abs,
                        bias=iota_ap(q),
                        scale=-1.0,
                    )
                # split the tent construction between DVE and GPSIMD: both
                # are SBUF-only ops and GPSIMD has spare capacity
                nc.vector.tensor_scalar(
                    out=tent[:, 0:2, :],
                    in0=dtmp[:, 0:2, :],
                    scalar1=1.0,
                    scalar2=0.0,
                    op0=mybir.AluOpType.subtract,
                    op1=mybir.AluOpType.min,
                )
                nc.gpsimd.tensor_scalar(
                    out=tent[:, 2:4, :],
                    in0=dtmp[:, 2:4, :],
                    scalar1=1.0,
                    scalar2=0.0,
                    op0=mybir.AluOpType.subtract,
                    op1=mybir.AluOpType.min,
                )
                return tent

            def start_gather(b):
                # one indirect DMA per row-window block; offsets [128,1] is
                # the only descriptor layout real SWDGE supports. Separate
                # tiles per block so stage-1 only waits on the slot it reads.
                views = []
                for t in range(3):
                    crop = crop_pool.tile([128, ROWE], F16, tag=f"crop{t}")
                    nc.gpsimd.indirect_dma_start(
                        out=crop[:],
                        out_offset=None,
                        in_=imgs[:, :],
                        in_offset=IndirectOffsetOnAxis(
                            ap=off_ap(b * 3 + t), axis=1
                        ),
                    )
                    # stride-3 channel view: [128, ch, col]
                    views.append(crop.rearrange("p (x c) -> p c x", c=C))
                return views

            # software pipeline: tents and gather run one image ahead
            tent_next = make_tents(0)
            crop_next = start_gather(0)
            for b in range(BL):
                tent, cviews = tent_next, crop_next
                if b + 1 < BL:
                    crop_next = start_gather(b + 1)
                    tent_next = make_tents(b + 1)

                out_sb = out_pool.tile([BI, 9, O], U8, tag="osb")

                def stage1(c):
                    # vertical resize, row-windowed
                    rt = rt_pool.tile([128, 3, O], F16, tag="rt")
                    for q in range(3):
                        pmm = ps1.tile([128, O], F32, tag="pmm")
                        for t in range(3):
                            nc.tensor.matmul(
                                pmm[:, BI * t : BI * (t + 1)],
                                cviews[t][:, c, 128 * q : 128 * (q + 1)],
                                tent[:, 0, BI * t : BI * (t + 1)],
                                start=True, stop=True,
                            )
                        copy_rot(rt[:, q, :], pmm[:], quant=False)
                    return rt

                def stage2(c, rt):
                    # horizontal resize + quantize
                    for T in range(3):
                        pm2 = ps2.tile([BI, O], F32, tag="pm2")
                        for q in range(3):
                            nc.tensor.matmul(
                                pm2[:],
                                rt[:, q, BI * T : BI * (T + 1)],
                                tent[:, 1 + q, :],
                                start=(q == 0), stop=(q == 2),
                            )
                        copy_rot(out_sb[:, 3 * c + T, :], pm2[:], quant=True)

                # all stage-1 passes first: by the time stage-2 needs an rt
                # tile its PSUM->SBUF copy has long drained
                rts = [stage1(c) for c in range(C)]
                for c in range(C):
                    stage2(c, rts[c])
                    # per-channel store so the tail only waits on channel 2
                    dst = AP(
                        tensor=out,
                        offset=(b * C + c) * BI * 3 * O,
                        ap=[[3 * O, BI], [O, 3], [1, O]],
                    )
                    nc.sync.dma_start(dst, out_sb[:, 3 * c : 3 * c + 3, :])

    nc.compile()
    return nc


def _axis_coords(pad, new_n, nf, lim):
    """Crop-local bilinear source coords, f32 math mirroring the reference."""
    f32 = np.float32
    i = np.arange(O, dtype=np.int64) - pad
    valid = (i >= 0) & (i < new_n)
    src = (i.astype(f32) + f32(0.5)) * nf
    src = src / f32(new_n)
    src = src - f32(0.5)
    src = np.clip(src, f32(0.0), f32(lim - 1))
    src[~valid] = INVALID
    return src.astype(np.float32), valid


def _host_params(images, boxes):
    """Per-core host prep. images: [BL,3,768,768] f32, boxes: [BL,4] i32.

    Returns (input map, per-image pad/dequant info).
    """
    import ml_dtypes

    f32 = np.float32
    offs = np.zeros((128, BL * 3), np.int32)
    coords = np.full((BL * 2 * O,), INVALID, np.float32)

    mn = float(images.min()) - 2.0
    mx = float(images.max()) + 2.0
    qa = (mx - mn) / 255.0
    qb = mn

    info = []
    p = np.arange(128)
    for b in range(BL):
        xb, yb, wb, hb = (int(v) for v in boxes[b])
        wf, hf = f32(wb), f32(hb)
        scale = f32(O) / np.maximum(wf, hf)
        new_w = int(np.round(wf * scale))
        new_h = int(np.round(hf * scale))
        pad_top = (O - new_h) // 2 if hb < wb else 0
        pad_left = (O - new_w) // 2 if hb >= wb else 0

        sy, vy = _axis_coords(pad_top, new_h, hf, hb)
        sx, vx = _axis_coords(pad_left, new_w, wf, wb)

        # per-block row windows: window [base_t, base_t+128) of IMAGE rows,
        # guaranteed inside the image, covering all taps of the block
        syw = sy.copy()
        for t in range(3):
            blk = slice(BI * t, BI * (t + 1))
            v = vy[blk]
            base = 0
            if v.any():
                s = sy[blk][v]
                lo = int(np.floor(s.min()))
                hi = min(int(np.floor(s.max())) + 1, hb - 1)
                assert hi - lo <= 127, (b, t, lo, hi)
                base = min(yb + lo, H - 128)
                syw[blk] = np.where(v, sy[blk] + f32(yb - base), INVALID)
            else:
                base = min(yb, H - 128)
            offs[:, b * 3 + t] = (((base + p) * W + xb) * C
                                  + b * IMG_ELEMS).astype(np.int32)

        base2 = b * 2 * O
        coords[base2 : base2 + O] = syw
        coords[base2 + O : base2 + 2 * O] = sx

        info.append((pad_top, new_h, pad_left, new_w))

    chi = coords.astype(ml_dtypes.bfloat16)
    clo = (coords - chi.astype(np.float32)).astype(ml_dtypes.bfloat16)

    # pk1: offs | iota (f32 bits) | quant scale+bias (f32 bits)
    pk1 = np.zeros((128, BL * 3 + 5), np.int32)
    pk1[:, 0 : BL * 3] = offs
    iota = (np.arange(128, dtype=np.float32)[:, None]
            + np.float32(128.0) * np.arange(3, dtype=np.float32)[None, :])
    pk1[:, BL * 3 : BL * 3 + 3] = iota.view(np.int32)
    qsb = np.empty((128, 2), np.float32)
    qsb[:, 0] = 1.0 / qa
    qsb[:, 1] = -qb / qa + 0.5  # +0.5: convert-to-uint8 truncates
    pk1[:, BL * 3 + 3 : BL * 3 + 5] = qsb.view(np.int32)

    # pk2: chi | clo | ones (all bf16)
    pk2 = np.empty((1, 2 * BL * 2 * O + 128), ml_dtypes.bfloat16)
    pk2[0, 0 : BL * 2 * O] = chi
    pk2[0, BL * 2 * O : 2 * BL * 2 * O] = clo
    pk2[0, 2 * BL * 2 * O :] = np.ones(128, ml_dtypes.bfloat16)

    in_map = dict(
        imgs=np.ascontiguousarray(
            images.transpose(0, 2, 3, 1)
        ).astype(np.float16).reshape(1, TOT),
        pk1=pk1,
        pk2=pk2,
    )
    return in_map, (qa, qb, info)


def kernel(images: np.ndarray, boxes: np.ndarray) -> np.ndarray:
    global _CACHED, LAST_RESULT
    if _CACHED is None:
        _CACHED = _build()
    nc = _CACHED

    prep = [
        _host_params(
            np.asarray(images[m * BL : (m + 1) * BL], dtype=np.float32),
            np.asarray(boxes[m * BL : (m + 1) * BL]),
        )
        for m in range(N_CORES)
    ]
    in_maps = [pm for pm, _ in prep]
    res = run_bass_kernel_spmd(nc, in_maps, core_ids=list(range(N_CORES)))
    LAST_RESULT = res

    full = np.empty((B, C, O, O), np.float32)
    for m in range(N_CORES):
        qa, qb, info = prep[m][1]
        raw = np.asarray(res.results[m]["out"])  # [BL, C, 112, 3, 336] uint8
        deq = raw.astype(np.float32) * np.float32(qa) + np.float32(qb)
        # row index i = 112*ic + p  ->  [BL, C, 336, 336]
        deq = deq.transpose(0, 1, 3, 2, 4).reshape(BL, C, O, O)
        for b in range(BL):
            pt, nh, pl, nw = info[b]
            g = m * BL + b
            full[g] = np.float32(127.0)
            full[g, :, pt : pt + nh, pl : pl + nw] = deq[
                b, :, pt : pt + nh, pl : pl + nw
            ]
    return full


# revision 53
# speedup vs baseline: 2.0233x; 1.2194x over previous
"""BoxCrop kernel for Trainium2 (8 NeuronCores, Bass/Tile).

Fused crop -> aspect-preserving bilinear resize (long side 336) -> square pad
(fill=127) for a batch of 64 images [64,3,768,768] with per-image XYWH boxes.

Strategy (data-parallel with shape-sorted slot assignment), v5:
- The host sorts the 64 images (tall boxes by width, wide boxes by height)
  and assigns sorted rank k to core k%8, slot k//8, so the 8 images sharing
  a slot have similar crop shapes. Per slot the kernel compiles with trimmed
  static shapes: nb = row blocks (ceil(max nh/112)), KC = gathered cols
  (max wb+1), nq = col chunks (ceil(KC/128)), N2 = output cols (max nw,
  16-aligned). Valid rows/cols are RE-BASED to start at 0 on the device;
  the host re-inserts pad offsets during reassembly.
- Host also converts images to fp16 channel-interleaved [H,W,C] layout and
  computes crop-local bilinear source coords (f32, clamped, invalid=-30000),
  per-block row windows (slope <= ~8/7 so each 112-output block sources
  <= 128 consecutive rows -- asserted), gather offsets, and uint8 quant
  params.
- Device per image:
    coords: rank-1 bf16 matmuls (hi+lo split) broadcast [1,336] -> PSUM.
    tents: Abs activation (bias = iota+128q) + tensor_scalar min(d-1,0)
      -> negated tents fp16; slot 0 = vertical A', slots 1+q = Wx chunks.
    gather: nb indirect DMAs, offsets [128,1] (the only HW-supported form),
      each descriptor = KC cols x 3 channels, contiguous fp16.
    stage 1: per (ch, chunk q): nb matmuls (contraction = 128-row window,
      N=112) -> PSUM; copy to SBUF fp16.
    stage 2: per (ch, i-tile T): nq accumulating matmuls (N=N2) -> PSUM;
      quantize-copy (scale,bias per-partition, Relu==identity) -> uint8.
    out DMA per (img,ch); host dequantizes, un-permutes, fills 127 pad.
- (negated tents: stage-1 and stage-2 negations cancel in the product)
"""
import numpy as np

import concourse.bacc as bacc
import concourse.bass as bass
import concourse.tile as tile
from concourse import mybir
from concourse.bass import AP, IndirectOffsetOnAxis
from concourse.bass_utils import run_bass_kernel_spmd

F32 = mybir.dt.float32
BF16 = mybir.dt.bfloat16
F16 = mybir.dt.float16
U8 = mybir.dt.uint8
I32 = mybir.dt.int32

N_CORES = 8
B = 64
BL = B // N_CORES          # images (slots) per core
C = 3
H = W = 768
O = 336                    # output size
BI = 112                   # output rows per block
IMG_ELEMS = C * H * W
TOT = BL * IMG_ELEMS
INVALID = np.float32(-30000.0)

_BUILDS = {}
_CACHED = None   # most recently used compiled module (for external tooling)
LAST_RESULT = None


def _build(shapes):
    """shapes: tuple of (nb, KC, N2) per slot."""
    nc = bacc.Bacc("TRN2", target_bir_lowering=False, debug=False)

    NGATH = sum(nb for nb, _, _ in shapes)
    imgs = nc.dram_tensor("imgs", [1, TOT], F16, kind="ExternalInput")
    # pk1: offs[0:NGATH] | iota (f32 bits) x3 | quant scale+bias (f32 bits)
    pk1 = nc.dram_tensor("pk1", [128, NGATH + 5], I32, kind="ExternalInput")
    # pk2: chi | clo | ones  (all bf16)
    NCO = BL * 2 * O
    pk2 = nc.dram_tensor("pk2", [1, 2 * NCO + 128], BF16, kind="ExternalInput")
    out = nc.dram_tensor("out", [BL, C, BI, 3, O], U8, kind="ExternalOutput")

    with tile.TileContext(nc) as tc:
        with (
            tc.tile_pool(name="const", bufs=1) as cpool,
            tc.tile_pool(name="crop", bufs=2) as crop_pool,
            tc.tile_pool(name="dtmp", bufs=2) as dtmp_pool,
            tc.tile_pool(name="tent", bufs=2) as tent_pool,
            tc.tile_pool(name="rt", bufs=4) as rt_pool,
            tc.tile_pool(name="osb", bufs=2) as out_pool,
            tc.tile_pool(name="pc", bufs=1, space="PSUM") as pc_pool,
            tc.tile_pool(name="ps1", bufs=3, space="PSUM") as ps1,
            tc.tile_pool(name="ps2", bufs=3, space="PSUM") as ps2,
        ):
            pk1_sb = cpool.tile([128, NGATH + 5], I32, tag="pk1")
            nc.sync.dma_start(pk1_sb[:], pk1[:])
            pk2_sb = cpool.tile([1, 2 * NCO + 128], BF16, tag="pk2")
            nc.scalar.dma_start(pk2_sb[:], pk2[:])

            def off_ap(col):
                return pk1_sb[:, col : col + 1]

            def iota_ap(q):
                return pk1_sb[:, NGATH + q : NGATH + q + 1].bitcast(F32)

            q_scale = pk1_sb[0:BI, NGATH + 3 : NGATH + 4].bitcast(F32)
            q_bias = pk1_sb[0:BI, NGATH + 4 : NGATH + 5].bitcast(F32)

            def chi_ap(sl):
                return pk2_sb[:, sl : sl + O]

            def clo_ap(sl):
                return pk2_sb[:, NCO + sl : NCO + sl + O]

            ones_sb = pk2_sb[:, 2 * NCO : 2 * NCO + 128]

            # PSUM->SBUF copies may only run on Act or DVE (GPSIMD cannot
            # access PSUM); Act also runs the Abs pass, so DVE gets more
            cp_engines = [1, 0, 1, 0, 1, 0, 1, 0, 1]  # 0=Act 1=DVE
            cp_idx = 0

            def copy_rot(dst, src, quant):
                nonlocal cp_idx
                e = cp_engines[cp_idx % len(cp_engines)]
                cp_idx += 1
                if quant:
                    if e == 0:
                        # quantized values are all > 0, so Relu == identity
                        # (Copy does not accept AP bias/scale)
                        nc.scalar.activation(
                            dst, src, mybir.ActivationFunctionType.Relu,
                            bias=q_bias, scale=q_scale,
                        )
                    else:
                        nc.vector.tensor_scalar(
                            out=dst, in0=src, scalar1=q_scale, scalar2=q_bias,
                            op0=mybir.AluOpType.mult, op1=mybir.AluOpType.add,
                        )
                else:
                    if e == 0:
                        nc.scalar.copy(dst, src)
                    else:
                        nc.vector.tensor_copy(dst, src)

            def make_tents(b, nq):
                """Broadcast sy/sx coords (bf16 hi+lo rank-1 matmuls), then
                Abs with shifted iota biases to derive the nq Wx chunks."""
                tent = tent_pool.tile([128, 4, O], F16, tag="tent")
                dtmp = dtmp_pool.tile([128, 4, O], F16, tag="dtmp")
                pc = pc_pool.tile([128, 2, 512], F32, tag="pc")
                for s in range(2):
                    sl = (b * 2 + s) * O
                    nc.tensor.matmul(
                        pc[:, s, 0:O], ones_sb, chi_ap(sl),
                        start=True, stop=False,
                    )
                    nc.tensor.matmul(
                        pc[:, s, 0:O], ones_sb, clo_ap(sl),
                        start=False, stop=True,
                    )
                # dtmp slots: 0 = |p - sy|, 1+q = |(128q + p) - sx|
                nc.scalar.activation(
                    dtmp[:, 0:2, :],
                    pc[:, :, 0:O],
                    mybir.ActivationFunctionType.Abs,
                    bias=iota_ap(0),
                    scale=-1.0,
                )
                for q in range(1, nq):
                    nc.scalar.activation(
                        dtmp[:, 1 + q, :],
                        pc[:, 1, 0:O],
                        mybir.ActivationFunctionType.Abs,
                        bias=iota_ap(q),
                        scale=-1.0,
                    )
                # tent ts split between DVE and the otherwise-idle GPSIMD
                nc.vector.tensor_scalar(
                    out=tent[:, 0:2, :], in0=dtmp[:, 0:2, :],
                    scalar1=1.0, scalar2=0.0,
                    op0=mybir.AluOpType.subtract, op1=mybir.AluOpType.min,
                )
                if nq > 1:
                    nc.gpsimd.tensor_scalar(
                        out=tent[:, 2 : 1 + nq, :], in0=dtmp[:, 2 : 1 + nq, :],
                        scalar1=1.0, scalar2=0.0,
                        op0=mybir.AluOpType.subtract, op1=mybir.AluOpType.min,
                    )
                return tent

            def start_gather(b, goff, nb, KC):
                views = []
                for t in range(nb):
                    crop = crop_pool.tile([128, KC * C], F16, tag=f"crop{t}")
                    nc.gpsimd.indirect_dma_start(
                        out=crop[:],
                        out_offset=None,
                        in_=imgs[:, :],
                        in_offset=IndirectOffsetOnAxis(
                            ap=off_ap(goff + t), axis=1
                        ),
                    )
                    views.append(crop.rearrange("p (x c) -> p c x", c=C))
                return views

            goffs = []
            g = 0
            for nb, KC, N2 in shapes:
                goffs.append(g)
                g += nb

            # software pipeline: tents and gather run one image ahead
            tent_next = make_tents(0, -(-shapes[0][1] // 128))
            crop_next = start_gather(0, goffs[0], shapes[0][0], shapes[0][1])
            for b in range(BL):
                nb, KC, N2 = shapes[b]
                nq = -(-KC // 128)
                tent, cviews = tent_next, crop_next
                if b + 1 < BL:
                    nb1, KC1, _ = shapes[b + 1]
                    crop_next = start_gather(b + 1, goffs[b + 1], nb1, KC1)
                    tent_next = make_tents(b + 1, -(-KC1 // 128))

                out_sb = out_pool.tile([BI, 9, O], U8, tag="osb")

                def stage1(c):
                    # vertical resize, row-windowed
                    rt = rt_pool.tile([128, 3, O], F16, tag="rt")
                    for q in range(nq):
                        M = min(128, KC - 128 * q)
                        pmm = ps1.tile([128, O], F32, tag="pmm")
                        for t in range(nb):
                            nc.tensor.matmul(
                                pmm[0:M, BI * t : BI * (t + 1)],
                                cviews[t][:, c, 128 * q : 128 * q + M],
                                tent[:, 0, BI * t : BI * (t + 1)],
                                start=True, stop=True,
                            )
                        copy_rot(
                            rt[0:M, q, 0 : BI * nb],
                            pmm[0:M, 0 : BI * nb],
                            quant=False,
                        )
                    return rt

                def stage2(c, rt):
                    # horizontal resize + quantize
                    for T in range(nb):
                        pm2 = ps2.tile([BI, O], F32, tag="pm2")
                        for q in range(nq):
                            M = min(128, KC - 128 * q)
                            nc.tensor.matmul(
                                pm2[:, 0:N2],
                                rt[0:M, q, BI * T : BI * (T + 1)],
                                tent[0:M, 1 + q, 0:N2],
                                start=(q == 0), stop=(q == nq - 1),
                            )
                        copy_rot(
                            out_sb[:, 3 * c + T, 0:N2],
                            pm2[:, 0:N2],
                            quant=True,
                        )

                rts = [stage1(c) for c in range(C)]
                for c in range(C):
                    stage2(c, rts[c])
                    dst = AP(
                        tensor=out,
                        offset=(b * C + c) * BI * 3 * O,
                        ap=[[3 * O, BI], [O, nb], [1, N2]],
                    )
                    nc.sync.dma_start(dst, out_sb[:, 3 * c : 3 * c + nb, 0:N2])

    nc.compile()
    return nc


def _axis_coords(pad, new_n, nf, lim):
    """Crop-local bilinear source coords, f32 math mirroring the reference."""
    f32 = np.float32
    i = np.arange(O, dtype=np.int64) - pad
    valid = (i >= 0) & (i < new_n)
    src = (i.astype(f32) + f32(0.5)) * nf
    src = src / f32(new_n)
    src = src - f32(0.5)
    src = np.clip(src, f32(0.0), f32(lim - 1))
    src[~valid] = INVALID
    return src.astype(np.float32), valid


def _plan(boxes):
    """Sorted slot assignment + per-slot shapes from the full box list."""
    f32 = np.float32
    geo = []
    for b in range(B):
        xb, yb, wb, hb = (int(v) for v in boxes[b])
        scale = f32(O) / np.maximum(f32(wb), f32(hb))
        nw = int(np.round(f32(wb) * scale))
        nh = int(np.round(f32(hb) * scale))
        geo.append((nh, nw, wb, hb))
    tall = sorted((i for i in range(B) if geo[i][0] >= geo[i][1]),
                  key=lambda i: -geo[i][2])
    wide = sorted((i for i in range(B) if geo[i][0] < geo[i][1]),
                  key=lambda i: -geo[i][0])
    perm = tall + wide  # rank k -> original index
    shapes = []
    for s in range(BL):
        slot = perm[8 * s : 8 * s + 8]
        nh = max(geo[i][0] for i in slot)
        nw = max(geo[i][1] for i in slot)
        wb = max(geo[i][2] for i in slot)
        nb = -(-nh // BI)
        KC = min(W // 2, ((wb + 1 + 7) // 8) * 8)
        N2 = min(O, ((nw + 15) // 16) * 16)
        shapes.append((nb, KC, N2))
    return perm, tuple(shapes), geo


def _host_params(images, boxes, shapes, geo_list):
    """Per-core host prep. images: [BL,3,768,768] f32 (already permuted)."""
    import ml_dtypes

    f32 = np.float32
    NGATH = sum(nb for nb, _, _ in shapes)
    offs = np.zeros((128, NGATH), np.int32)
    coords = np.full((BL * 2 * O,), INVALID, np.float32)

    mn = float(images.min()) - 2.0
    mx = float(images.max()) + 2.0
    qa = (mx - mn) / 255.0
    qb = mn

    info = []
    p = np.arange(128)
    goff = 0
    for b in range(BL):
        nb, KC, N2 = shapes[b]
        nh_s, nw_s, wb_s, hb_s = geo_list[b]
        xb, yb, wb, hb = (int(v) for v in boxes[b])
        wf, hf = f32(wb), f32(hb)
        scale = f32(O) / np.maximum(wf, hf)
        new_w = int(np.round(wf * scale))
        new_h = int(np.round(hf * scale))
        pad_top = (O - new_h) // 2 if hb < wb else 0
        pad_left = (O - new_w) // 2 if hb >= wb else 0

        sy, vy = _axis_coords(pad_top, new_h, hf, hb)
        sx, vx = _axis_coords(pad_left, new_w, wf, wb)

        # re-base: device row i' = i - pad_top, device col j' = j - pad_left
        syr = np.full(O, INVALID, np.float32)
        syr[0:new_h] = sy[pad_top : pad_top + new_h]
        sxr = np.full(O, INVALID, np.float32)
        sxr[0:new_w] = sx[pad_left : pad_left + new_w]
        assert new_h <= nb * BI and new_w <= N2

        # per-block row windows over the re-based rows
        syw = syr.copy()
        for t in range(nb):
            blk = slice(BI * t, BI * (t + 1))
            v = syr[blk] > INVALID / 2
            base = min(yb, H - 128)
            if v.any():
                s = syr[blk][v]
                lo = int(np.floor(s.min()))
                hi = min(int(np.floor(s.max())) + 1, hb - 1)
                assert hi - lo <= 127, (b, t, lo, hi)
                base = min(yb + lo, H - 128)
                syw[blk] = np.where(v, syr[blk] + f32(yb - base), INVALID)
            offs[:, goff + t] = (((base + p) * W + xb) * C
                                 + b * IMG_ELEMS).astype(np.int32)
        goff += nb

        base2 = b * 2 * O
        coords[base2 : base2 + O] = syw
        coords[base2 + O : base2 + 2 * O] = sxr

        info.append((pad_top, new_h, pad_left, new_w))

    chi = coords.astype(ml_dtypes.bfloat16)
    clo = (coords - chi.astype(np.float32)).astype(ml_dtypes.bfloat16)

    pk1 = np.zeros((128, NGATH + 5), np.int32)
    pk1[:, 0:NGATH] = offs
    iota = (np.arange(128, dtype=np.float32)[:, None]
            + np.float32(128.0) * np.arange(3, dtype=np.float32)[None, :])
    pk1[:, NGATH : NGATH + 3] = iota.view(np.int32)
    qsb = np.empty((128, 2), np.float32)
    qsb[:, 0] = 1.0 / qa
    qsb[:, 1] = -qb / qa + 0.5  # +0.5: convert-to-uint8 truncates
    pk1[:, NGATH + 3 : NGATH + 5] = qsb.view(np.int32)

    NCO = BL * 2 * O
    pk2 = np.empty((1, 2 * NCO + 128), ml_dtypes.bfloat16)
    pk2[0, 0:NCO] = chi
    pk2[0, NCO : 2 * NCO] = clo
    pk2[0, 2 * NCO :] = np.ones(128, ml_dtypes.bfloat16)

    in_map = dict(
        imgs=np.ascontiguousarray(
            images.transpose(0, 2, 3, 1)
        ).astype(np.float16).reshape(1, TOT),
        pk1=pk1,
        pk2=pk2,
    )
    return in_map, (qa, qb, info)


def kernel(images: np.ndarray, boxes: np.ndarray) -> np.ndarray:
    global LAST_RESULT, _CACHED
    images = np.asarray(images, dtype=np.float32)
    boxes = np.asarray(boxes)

    perm, shapes, geo = _plan(boxes)
    if shapes not in _BUILDS:
        _BUILDS[shapes] = _build(shapes)
    nc = _CACHED = _BUILDS[shapes]

    prep = []
    for m in range(N_CORES):
        idx = [perm[8 * s + m] for s in range(BL)]
        prep.append(
            _host_params(
                images[idx], boxes[idx], shapes,
                [geo[i] for i in idx],
            )
        )
    in_maps = [pm for pm, _ in prep]
    res = run_bass_kernel_spmd(nc, in_maps, core_ids=list(range(N_CORES)))
    LAST_RESULT = res

    full = np.empty((B, C, O, O), np.float32)
    for m in range(N_CORES):
        qa, qb, info = prep[m][1]
        raw = np.asarray(res.results[m]["out"])  # [BL, C, 112, 3, 336] uint8
        deq = raw.astype(np.float32) * np.float32(qa) + np.float32(qb)
        # device row i' = 112*ic + p -> [BL, C, 336, 336] (re-based)
        deq = deq.transpose(0, 1, 3, 2, 4).reshape(BL, C, O, O)
        for s in range(BL):
            pt, nh, pl, nw = info[s]
            g = perm[8 * s + m]
            full[g] = np.float32(127.0)
            full[g, :, pt : pt + nh, pl : pl + nw] = deq[s, :, 0:nh, 0:nw]
    return full


# revision 54
# speedup vs baseline: 2.0461x; 1.0113x over previous
"""BoxCrop kernel for Trainium2 (8 NeuronCores, Bass/Tile).

Fused crop -> aspect-preserving bilinear resize (long side 336) -> square pad
(fill=127) for a batch of 64 images [64,3,768,768] with per-image XYWH boxes.

Strategy (data-parallel with shape-sorted slot assignment), v5:
- The host sorts the 64 images (tall boxes by width, wide boxes by height)
  and assigns sorted rank k to core k%8, slot k//8, so the 8 images sharing
  a slot have similar crop shapes. Per slot the kernel compiles with trimmed
  static shapes: nb = row blocks (ceil(max nh/112)), KC = gathered cols
  (max wb+1), nq = col chunks (ceil(KC/128)), N2 = output cols (max nw,
  16-aligned). Valid rows/cols are RE-BASED to start at 0 on the device;
  the host re-inserts pad offsets during reassembly.
- Host also converts images to fp16 channel-interleaved [H,W,C] layout and
  computes crop-local bilinear source coords (f32, clamped, invalid=-30000),
  per-block row windows (slope <= ~8/7 so each 112-output block sources
  <= 128 consecutive rows -- asserted), gather offsets, and uint8 quant
  params.
- Device per image:
    coords: rank-1 bf16 matmuls (hi+lo split) broadcast [1,336] -> PSUM.
    tents: Abs activation (bias = iota+128q) + tensor_scalar min(d-1,0)
      -> negated tents fp16; slot 0 = vertical A', slots 1+q = Wx chunks.
    gather: nb indirect DMAs, offsets [128,1] (the only HW-supported form),
      each descriptor = KC cols x 3 channels, contiguous fp16.
    stage 1: per (ch, chunk q): nb matmuls (contraction = 128-row window,
      N=112) -> PSUM; copy to SBUF fp16.
    stage 2: per (ch, i-tile T): nq accumulating matmuls (N=N2) -> PSUM;
      quantize-copy (scale,bias per-partition, Relu==identity) -> uint8.
    out DMA per (img,ch); host dequantizes, un-permutes, fills 127 pad.
- (negated tents: stage-1 and stage-2 negations cancel in the product)
"""
import numpy as np

import concourse.bacc as bacc
import concourse.bass as bass
import concourse.tile as tile
from concourse import mybir
from concourse.bass import AP, IndirectOffsetOnAxis
from concourse.bass_utils import run_bass_kernel_spmd

F32 = mybir.dt.float32
BF16 = mybir.dt.bfloat16
F16 = mybir.dt.float16
U8 = mybir.dt.uint8
I32 = mybir.dt.int32

N_CORES = 8
B = 64
BL = B // N_CORES          # images (slots) per core
C = 3
H = W = 768
O = 336                    # output size
BI = 112                   # output rows per block
IMG_ELEMS = C * H * W
TOT = BL * IMG_ELEMS
INVALID = np.float32(-30000.0)

_BUILDS = {}
_CACHED = None   # most recently used compiled module (for external tooling)
LAST_RESULT = None


def _build(shapes):
    """shapes: tuple of (nb, KC, N2) per slot."""
    nc = bacc.Bacc("TRN2", target_bir_lowering=False, debug=False)

    NGATH = sum(nb for nb, _, _ in shapes)
    imgs = nc.dram_tensor("imgs", [1, TOT], F16, kind="ExternalInput")
    # pk1: offs[0:NGATH] | iota (f32 bits) x3 | quant scale+bias (f32 bits)
    pk1 = nc.dram_tensor("pk1", [128, NGATH + 5], I32, kind="ExternalInput")
    # pk2: chi | clo | ones  (all bf16)
    NCO = BL * 2 * O
    pk2 = nc.dram_tensor("pk2", [1, 2 * NCO + 128], BF16, kind="ExternalInput")
    out = nc.dram_tensor("out", [BL, C, BI, 3, O], U8, kind="ExternalOutput")

    with tile.TileContext(nc) as tc:
        with (
            tc.tile_pool(name="const", bufs=1) as cpool,
            tc.tile_pool(name="crop", bufs=2) as crop_pool,
            tc.tile_pool(name="dtmp", bufs=2) as dtmp_pool,
            tc.tile_pool(name="tent", bufs=2) as tent_pool,
            tc.tile_pool(name="rt", bufs=4) as rt_pool,
            tc.tile_pool(name="osb", bufs=2) as out_pool,
            tc.tile_pool(name="pc", bufs=2, space="PSUM") as pc_pool,
            tc.tile_pool(name="ps1", bufs=3, space="PSUM") as ps1,
            tc.tile_pool(name="ps2", bufs=3, space="PSUM") as ps2,
        ):
            pk1_sb = cpool.tile([128, NGATH + 5], I32, tag="pk1")
            nc.sync.dma_start(pk1_sb[:], pk1[:])
            pk2_sb = cpool.tile([1, 2 * NCO + 128], BF16, tag="pk2")
            nc.scalar.dma_start(pk2_sb[:], pk2[:])

            def off_ap(col):
                return pk1_sb[:, col : col + 1]

            def iota_ap(q):
                return pk1_sb[:, NGATH + q : NGATH + q + 1].bitcast(F32)

            q_scale = pk1_sb[0:BI, NGATH + 3 : NGATH + 4].bitcast(F32)
            q_bias = pk1_sb[0:BI, NGATH + 4 : NGATH + 5].bitcast(F32)

            def chi_ap(sl):
                return pk2_sb[:, sl : sl + O]

            def clo_ap(sl):
                return pk2_sb[:, NCO + sl : NCO + sl + O]

            ones_sb = pk2_sb[:, 2 * NCO : 2 * NCO + 128]

            # PSUM->SBUF copies may only run on Act or DVE (GPSIMD cannot
            # access PSUM); Act also runs the Abs pass, so DVE gets more
            cp_engines = [1, 0, 1, 0, 1, 0, 1, 0, 1]  # 0=Act 1=DVE
            cp_idx = 0

            def copy_rot(dst, src, quant):
                nonlocal cp_idx
                e = cp_engines[cp_idx % len(cp_engines)]
                cp_idx += 1
                if quant:
                    if e == 0:
                        # quantized values are all > 0, so Relu == identity
                        # (Copy does not accept AP bias/scale)
                        nc.scalar.activation(
                            dst, src, mybir.ActivationFunctionType.Relu,
                            bias=q_bias, scale=q_scale,
                        )
                    else:
                        nc.vector.tensor_scalar(
                            out=dst, in0=src, scalar1=q_scale, scalar2=q_bias,
                            op0=mybir.AluOpType.mult, op1=mybir.AluOpType.add,
                        )
                else:
                    if e == 0:
                        nc.scalar.copy(dst, src)
                    else:
                        nc.vector.tensor_copy(dst, src)

            def make_tents(b, nq):
                """Broadcast sy/sx coords (bf16 hi+lo rank-1 matmuls), then
                Abs with shifted iota biases to derive the nq Wx chunks."""
                tent = tent_pool.tile([128, 4, O], F16, tag="tent")
                dtmp = dtmp_pool.tile([128, 4, O], F16, tag="dtmp")
                # two 1-bank coord tiles (bufs=2): image b+1's sy
                # broadcast can start while image b's sx Abs still reads
                pcs = []
                for s in range(2):
                    pc = pc_pool.tile([128, 512], F32, tag="pc")
                    sl = (b * 2 + s) * O
                    nc.tensor.matmul(
                        pc[0:128, 0:O], ones_sb, chi_ap(sl),
                        start=True, stop=False,
                    )
                    nc.tensor.matmul(
                        pc[0:128, 0:O], ones_sb, clo_ap(sl),
                        start=False, stop=True,
                    )
                    pcs.append(pc)
                # dtmp slots: 0 = |p - sy|, 1+q = |(128q + p) - sx|
                nc.scalar.activation(
                    dtmp[:, 0, :],
                    pcs[0][0:128, 0:O],
                    mybir.ActivationFunctionType.Abs,
                    bias=iota_ap(0),
                    scale=-1.0,
                )
                for q in range(nq):
                    nc.scalar.activation(
                        dtmp[:, 1 + q, :],
                        pcs[1][0:128, 0:O],
                        mybir.ActivationFunctionType.Abs,
                        bias=iota_ap(q),
                        scale=-1.0,
                    )
                # tent ts split between DVE and the otherwise-idle GPSIMD
                nc.vector.tensor_scalar(
                    out=tent[:, 0:2, :], in0=dtmp[:, 0:2, :],
                    scalar1=1.0, scalar2=0.0,
                    op0=mybir.AluOpType.subtract, op1=mybir.AluOpType.min,
                )
                if nq > 1:
                    nc.gpsimd.tensor_scalar(
                        out=tent[:, 2 : 1 + nq, :], in0=dtmp[:, 2 : 1 + nq, :],
                        scalar1=1.0, scalar2=0.0,
                        op0=mybir.AluOpType.subtract, op1=mybir.AluOpType.min,
                    )
                return tent

            def start_gather(b, goff, nb, KC):
                views = []
                for t in range(nb):
                    crop = crop_pool.tile([128, KC * C], F16, tag=f"crop{t}")
                    nc.gpsimd.indirect_dma_start(
                        out=crop[:],
                        out_offset=None,
                        in_=imgs[:, :],
                        in_offset=IndirectOffsetOnAxis(
                            ap=off_ap(goff + t), axis=1
                        ),
                    )
                    views.append(crop.rearrange("p (x c) -> p c x", c=C))
                return views

            goffs = []
            g = 0
            for nb, KC, N2 in shapes:
                goffs.append(g)
                g += nb

            # software pipeline: tents and gather run one image ahead
            tent_next = make_tents(0, -(-shapes[0][1] // 128))
            crop_next = start_gather(0, goffs[0], shapes[0][0], shapes[0][1])
            for b in range(BL):
                nb, KC, N2 = shapes[b]
                nq = -(-KC // 128)
                tent, cviews = tent_next, crop_next
                if b + 1 < BL:
                    nb1, KC1, _ = shapes[b + 1]
                    crop_next = start_gather(b + 1, goffs[b + 1], nb1, KC1)
                    tent_next = make_tents(b + 1, -(-KC1 // 128))

                out_sb = out_pool.tile([BI, 9, O], U8, tag="osb")

                def stage1(c):
                    # vertical resize, row-windowed
                    rt = rt_pool.tile([128, 3, O], F16, tag="rt")
                    for q in range(nq):
                        M = min(128, KC - 128 * q)
                        pmm = ps1.tile([128, O], F32, tag="pmm")
                        for t in range(nb):
                            nc.tensor.matmul(
                                pmm[0:M, BI * t : BI * (t + 1)],
                                cviews[t][:, c, 128 * q : 128 * q + M],
                                tent[:, 0, BI * t : BI * (t + 1)],
                                start=True, stop=True,
                            )
                        copy_rot(
                            rt[0:M, q, 0 : BI * nb],
                            pmm[0:M, 0 : BI * nb],
                            quant=False,
                        )
                    return rt

                def stage2(c, rt):
                    # horizontal resize + quantize
                    for T in range(nb):
                        pm2 = ps2.tile([BI, O], F32, tag="pm2")
                        for q in range(nq):
                            M = min(128, KC - 128 * q)
                            nc.tensor.matmul(
                                pm2[:, 0:N2],
                                rt[0:M, q, BI * T : BI * (T + 1)],
                                tent[0:M, 1 + q, 0:N2],
                                start=(q == 0), stop=(q == nq - 1),
                            )
                        copy_rot(
                            out_sb[:, 3 * c + T, 0:N2],
                            pm2[:, 0:N2],
                            quant=True,
                        )

                rts = [stage1(c) for c in range(C)]
                for c in range(C):
                    stage2(c, rts[c])
                    dst = AP(
                        tensor=out,
                        offset=(b * C + c) * BI * 3 * O,
                        ap=[[3 * O, BI], [O, nb], [1, N2]],
                    )
                    nc.sync.dma_start(dst, out_sb[:, 3 * c : 3 * c + nb, 0:N2])

    nc.compile()
    return nc


def _axis_coords(pad, new_n, nf, lim):
    """Crop-local bilinear source coords, f32 math mirroring the reference."""
    f32 = np.float32
    i = np.arange(O, dtype=np.int64) - pad
    valid = (i >= 0) & (i < new_n)
    src = (i.astype(f32) + f32(0.5)) * nf
    src = src / f32(new_n)
    src = src - f32(0.5)
    src = np.clip(src, f32(0.0), f32(lim - 1))
    src[~valid] = INVALID
    return src.astype(np.float32), valid


def _plan(boxes):
    """Sorted slot assignment + per-slot shapes from the full box list."""
    f32 = np.float32
    geo = []
    for b in range(B):
        xb, yb, wb, hb = (int(v) for v in boxes[b])
        scale = f32(O) / np.maximum(f32(wb), f32(hb))
        nw = int(np.round(f32(wb) * scale))
        nh = int(np.round(f32(hb) * scale))
        geo.append((nh, nw, wb, hb))
    tall = sorted((i for i in range(B) if geo[i][0] >= geo[i][1]),
                  key=lambda i: -geo[i][2])
    wide = sorted((i for i in range(B) if geo[i][0] < geo[i][1]),
                  key=lambda i: -geo[i][0])
    perm = tall + wide  # rank k -> original index
    shapes = []
    for s in range(BL):
        slot = perm[8 * s : 8 * s + 8]
        nh = max(geo[i][0] for i in slot)
        nw = max(geo[i][1] for i in slot)
        wb = max(geo[i][2] for i in slot)
        nb = -(-nh // BI)
        KC = min(W // 2, ((wb + 1 + 7) // 8) * 8)
        N2 = min(O, ((nw + 15) // 16) * 16)
        shapes.append((nb, KC, N2))
    return perm, tuple(shapes), geo


def _host_params(images, boxes, shapes, geo_list):
    """Per-core host prep. images: [BL,3,768,768] f32 (already permuted)."""
    import ml_dtypes

    f32 = np.float32
    NGATH = sum(nb for nb, _, _ in shapes)
    offs = np.zeros((128, NGATH), np.int32)
    coords = np.full((BL * 2 * O,), INVALID, np.float32)

    mn = float(images.min()) - 2.0
    mx = float(images.max()) + 2.0
    qa = (mx - mn) / 255.0
    qb = mn

    info = []
    p = np.arange(128)
    goff = 0
    for b in range(BL):
        nb, KC, N2 = shapes[b]
        nh_s, nw_s, wb_s, hb_s = geo_list[b]
        xb, yb, wb, hb = (int(v) for v in boxes[b])
        wf, hf = f32(wb), f32(hb)
        scale = f32(O) / np.maximum(wf, hf)
        new_w = int(np.round(wf * scale))
        new_h = int(np.round(hf * scale))
        pad_top = (O - new_h) // 2 if hb < wb else 0
        pad_left = (O - new_w) // 2 if hb >= wb else 0

        sy, vy = _axis_coords(pad_top, new_h, hf, hb)
        sx, vx = _axis_coords(pad_left, new_w, wf, wb)

        # re-base: device row i' = i - pad_top, device col j' = j - pad_left
        syr = np.full(O, INVALID, np.float32)
        syr[0:new_h] = sy[pad_top : pad_top + new_h]
        sxr = np.full(O, INVALID, np.float32)
        sxr[0:new_w] = sx[pad_left : pad_left + new_w]
        assert new_h <= nb * BI and new_w <= N2

        # per-block row windows over the re-based rows
        syw = syr.copy()
        for t in range(nb):
            blk = slice(BI * t, BI * (t + 1))
            v = syr[blk] > INVALID / 2
            base = min(yb, H - 128)
            if v.any():
                s = syr[blk][v]
                lo = int(np.floor(s.min()))
                hi = min(int(np.floor(s.max())) + 1, hb - 1)
                assert hi - lo <= 127, (b, t, lo, hi)
                base = min(yb + lo, H - 128)
                syw[blk] = np.where(v, syr[blk] + f32(yb - base), INVALID)
            offs[:, goff + t] = (((base + p) * W + xb) * C
                                 + b * IMG_ELEMS).astype(np.int32)
        goff += nb

        base2 = b * 2 * O
        coords[base2 : base2 + O] = syw
        coords[base2 + O : base2 + 2 * O] = sxr

        info.append((pad_top, new_h, pad_left, new_w))

    chi = coords.astype(ml_dtypes.bfloat16)
    clo = (coords - chi.astype(np.float32)).astype(ml_dtypes.bfloat16)

    pk1 = np.zeros((128, NGATH + 5), np.int32)
    pk1[:, 0:NGATH] = offs
    iota = (np.arange(128, dtype=np.float32)[:, None]
            + np.float32(128.0) * np.arange(3, dtype=np.float32)[None, :])
    pk1[:, NGATH : NGATH + 3] = iota.view(np.int32)
    qsb = np.empty((128, 2), np.float32)
    qsb[:, 0] = 1.0 / qa
    qsb[:, 1] = -qb / qa + 0.5  # +0.5: convert-to-uint8 truncates
    pk1[:, NGATH + 3 : NGATH + 5] = qsb.view(np.int32)

    NCO = BL * 2 * O
    pk2 = np.empty((1, 2 * NCO + 128), ml_dtypes.bfloat16)
    pk2[0, 0:NCO] = chi
    pk2[0, NCO : 2 * NCO] = clo
    pk2[0, 2 * NCO :] = np.ones(128, ml_dtypes.bfloat16)

    in_map = dict(
        imgs=np.ascontiguousarray(
            images.transpose(0, 2, 3, 1)
        ).astype(np.float16).reshape(1, TOT),
        pk1=pk1,
        pk2=pk2,
    )
    return in_map, (qa, qb, info)


def kernel(images: np.ndarray, boxes: np.ndarray) -> np.ndarray:
    global LAST_RESULT, _CACHED
    images = np.asarray(images, dtype=np.float32)
    boxes = np.asarray(boxes)

    perm, shapes, geo = _plan(boxes)
    if shapes not in _BUILDS:
        _BUILDS[shapes] = _build(shapes)
    nc = _CACHED = _BUILDS[shapes]

    prep = []
    for m in range(N_CORES):
        idx = [perm[8 * s + m] for s in range(BL)]
        prep.append(
            _host_params(
                images[idx], boxes[idx], shapes,
                [geo[i] for i in idx],
            )
        )
    in_maps = [pm for pm, _ in prep]
    res = run_bass_kernel_spmd(nc, in_maps, core_ids=list(range(N_CORES)))
    LAST_RESULT = res

    full = np.empty((B, C, O, O), np.float32)
    for m in range(N_CORES):
        qa, qb, info = prep[m][1]
        raw = np.asarray(res.results[m]["out"])  # [BL, C, 112, 3, 336] uint8
        deq = raw.astype(np.float32) * np.float32(qa) + np.float32(qb)
        # device row i' = 112*ic + p -> [BL, C, 336, 336] (re-based)
        deq = deq.transpose(0, 1, 3, 2, 4).reshape(BL, C, O, O)
        for s in range(BL):
            pt, nh, pl, nw = info[s]
            g = perm[8 * s + m]
            full[g] = np.float32(127.0)
            full[g, :, pt : pt + nh, pl : pl + nw] = deq[s, :, 0:nh, 0:nw]
    return full
